# revision 61
# baseline (speedup 1.0000x reference)
"""Trainium2 Bass kernel for AxisLengthNetMetric (chamfer-distance + L1-size metric).

Reference computation (per row n of N = 262144):
  gt_box row -> size (cols 3:6), rx (6:9), ry (9:12)
  rx_hat = rx/|rx|, ry_hat = ry/|ry|, rz = cross(rx_hat, ry_hat)
  corners u_c = sum_k sign[c,k] * 0.5*size[k] * axis_k   (8 corners, +-pairs)
  chamfer(corners, pred_pts[n]): d[p,q] = |a_p - b_q|^2, dist1 = min_q, dist2 = min_p
  out[0] = mean over (N,8) of dist1+dist2 ; out[1] = mean |size - pred_size|

Kernel strategy (v6):
- data parallel over 8 cores; per core 32768 rows as 128 partitions x 256.
- 4 distinct corners up to sign (u' = 2u prescaled, g' = u'.b):
    dist1 sums: min_q(b2 -+ g'), dist2: b2_q + min_i(a2_i - |g'|);
    a2/b2/min contributions accumulated via ACT accum_out, host-combined.
- t2pair: rz = rx_hat x ry_hat is orthogonal to both axes, so corner
  norms take only TWO values (|u0|=|u2|=A+, |u1|=|u3|=A-) and
  dist2 = b2 + min(A+ - max(|g0|,|g2|), A- - max(|g1|,|g3|)); the max
  runs on packed lane pairs, cutting 16 DVE elems/row off the t2 path.
- fp16 end to end (inputs DMA'd as fp16): every TensorTensor with all
  operands 2-byte and packed-innermost runs in DVE 2x mode (0.52 ns/elem).
- dot products as ONE DVE mul in [i,q,d] layout (d innermost => both
  broadcast operands stay packed => 2x), then the d-sum (the only work the
  gpsimd engine gets) as two flattened-3D adds on Pool; everything else
  rides DVE (2x) or ACT (abs/squares/copies/accumulations, transposed
  writes for free axis swaps).
- walrus constraints honored: no ScalarTensorTensor on Pool, no gpsimd
  min, no tensor_tensor_reduce; ScalarTensorTensor APs <= 3D.
- phase-1 software-pipelined (5-stage skew) over TILE_SIZES rows: the
  small head tile fills the tile-0 d-sum latency bubble, the small tail
  tile shortens the drain chain. phase-0 in 2 chunks [88,168] aligned to
  the tile-0/1 boundary; chunk-0's squares / rx,ry staging copies / cce
  expansion run on DVE (sq_dve, rxy_dve_c0, cce_dve_c0) to shorten the
  DMA->uta startup chain and fill DVE bubbles there; tile-0's b2 sums and
  l1d and chunk-0's cross products (which need only raw gt data)
  ride the Pool engine's idle pre-d-sum head window; the last tile's b2
  sums ride Pool's idle post-d-sum tail window (b2_gp_tail); chunk-0's
  corner combos + a2 sums (utc/a2s_gp_c0) fill the rest of the head
  window, leaving Pool saturated back-to-back from 12.7us to 33.8us.
- dump_last/dump_n: the last 3 tiles DMA raw St/l1d values (host sums
  them), removing their ACT accumulates from the drain path; the accT DMA
  is issued ahead of the final dump so the two DMA pipelines overlap
  instead of serializing on the SP queue; the last tile's final St min is
  split by row-halves so the dump trails the later branch by a half-op.
- engine busy: DVE ~50.7us, Pool ~41us, ACT ~28us; TimelineSim
  ~58.3us/core (v5: 65.3us, v4: 81.9us).
"""

import numpy as np

import concourse.bacc as bacc
import concourse.bass as bass  # noqa: F401
import concourse.tile as tile
from concourse import mybir

F32 = mybir.dt.float32
F16 = mybir.dt.float16
ALU = mybir.AluOpType
ACTF = mybir.ActivationFunctionType
AX = mybir.AxisListType

P = 128
N_CORES = 8
N_TOTAL = 262144
NC_N = N_TOTAL // N_CORES  # 32768 rows per core
G_PROD = 64                # rows per partition per heavy tile
# engine assignment knobs: True = put op on gpsimd (Pool)
KNOB = {
    "wtutc_gp": False, "xe_bufs": 2, "order": "rev", "p0k": 1, "nchunk": 2,
    "seq": False, "dsum": "pp", "p0_gp": False, "l1d_gp": False,
    "eebp_gp": False, "b2_gp": False, "t2b_gp": False,
    "t2pair": True, "dump_last": True, "rxy_dve_c0": True,
    "sq_dve": True, "chunks": [88, 168], "sched": "gen0c",
    "cce_dve_c0": True, "b2_gp_head": 1, "l1d_gp_head": 1,
    "cross_gp_c0": True, "b2_gp_tail": True, "dump_n": 3,
    "st_split_tail": True, "utc_gp_c0": True, "a2s_gp_c0": True,
}

# accT slots per tile
MINSUM, SQA, SQB, L1, T2SUM = 0, 1, 2, 3, 4
NSLOT = 5


TILE_SIZES = [28, 60, 72, 72, 24]  # shrinking tail shortens the drain chain


def build_nc(nc_n=NC_N, G=G_PROD, tile_sizes=None):
    GA = nc_n // P             # all rows per partition
    if tile_sizes is None:
        tile_sizes = TILE_SIZES if GA == 256 else [G] * (GA // G)
    assert sum(tile_sizes) == GA
    TILES = []
    r = 0
    for gsz in tile_sizes:
        TILES.append((r, gsz))
        r += gsz
    ntiles = len(TILES)

    nc = bacc.Bacc("TRN2", target_bir_lowering=False, debug=False)

    gt = nc.dram_tensor("gt", [nc_n, 12], F16, kind="ExternalInput").ap()
    pred = nc.dram_tensor("pred", [nc_n, 24], F16, kind="ExternalInput").ap()
    ps = nc.dram_tensor("ps", [nc_n, 3], F16, kind="ExternalInput").ap()
    out = nc.dram_tensor("out", [P, ntiles * NSLOT], F32, kind="ExternalOutput").ap()
    # last tile dumps raw St/l1d (host sums them): the final ACT accum +
    # its sem hop come off the drain critical path
    DUMP = KNOB.get("dump_last", False)
    DUMP_N = KNOB.get("dump_n", 1) if DUMP else 0
    outd = {}
    for td in range(ntiles - DUMP_N, ntiles):
        outd[td] = nc.dram_tensor(
            f"outd{td}", [P, TILES[td][1] * 19], F16, kind="ExternalOutput"
        ).ap()

    gt_r = gt.rearrange("(p g) f -> p g f", p=P)
    pred_r = pred.rearrange("(p g) f -> p g f", p=P)
    ps_r = ps.rearrange("(p g) f -> p g f", p=P)

    def gp_tt(out_, a, b, op):
        # plain TensorTensor on gpsimd (walrus rejects TensorScalarPtr on Pool)
        nc.gpsimd.tensor_tensor(out_, a, b, op=op)

    with tile.TileContext(nc) as tc:
        with (
            tc.tile_pool(name="per", bufs=1) as per,   # persistent / phase-0
            tc.tile_pool(name="io", bufs=KNOB.get("io_bufs", 3)) as io,
            tc.tile_pool(name="scr", bufs=KNOB.get("scr_bufs", 2)) as scr,
            tc.tile_pool(name="xe", bufs=KNOB["xe_bufs"]) as xe,
        ):
            HW = KNOB.get("halfw", False)  # dots vs basis {v0+v1, v0-v1, v2}
            accT = per.tile([P, ntiles, NSLOT], F32)
            # SQA only fills NCHUNK of the ntiles slots - zero the rest
            nc.gpsimd.memset(accT, 0.0)

            # warm the ACT function tables before any data dependency
            warm = per.tile([P, 2], F32)
            nc.vector.memset(warm, 1.0)
            for fn in (ACTF.Sqrt, ACTF.Square, ACTF.Abs, ACTF.Identity):
                nc.scalar.activation(warm[:, 0:1], warm[:, 1:2], fn)

            # ================= phase 0: corner basis (pipelined stage) ========
            gta = per.tile([P, GA, 12], F16)
            uta = per.tile([P, GA, 4, 3], F16)
            a2ba = per.tile([P, GA, 4], F16)
            basa = None  # [p, m, v2] basis per row when halfw
            if HW:
                basa = per.tile([P, GA, 3, 3], F16, name="basa")
            dumpts = {}
            for td in outd:
                dumpts[td] = per.tile(
                    [P, TILES[td][1], 19], F16, name=f"dumpt{td}"
                )
            CH = KNOB.get("chunks", "half")
            if isinstance(CH, (list, tuple)):
                CHUNKS = []
                r0c = 0
                for gc in CH:
                    CHUNKS.append((r0c, gc))
                    r0c += gc
                assert r0c == GA
            elif CH == "half":
                CHUNKS = [(0, GA // 2), (GA // 2, GA // 2)]
            elif CH == "64rest":
                CHUNKS = [(0, 64), (64, GA - 64)]
            elif CH == "3way":
                CHUNKS = [(0, 64), (64, 96), (160, GA - 160)]
            st0 = {}

            def P0a(c):
                r0, GC = CHUNKS[c]
                cs = slice(r0, r0 + GC)
                gtc = gta[:, cs]
                if c == 0 and KNOB.get("dma_split", False):
                    # split the first gt DMA + squares so the startup scalar
                    # chain begins after half the transfer
                    h = GC // 2
                    nc.sync.dma_start(out=gtc[:, 0:h], in_=gt_r[:, r0:r0 + h])
                    nc.sync.dma_start(out=gtc[:, h:GC], in_=gt_r[:, r0 + h:r0 + GC])
                else:
                    nc.sync.dma_start(out=gtc, in_=gt_r[:, cs])

                n2a = scr.tile([P, GC, 2], F16, tag="n2a")
                n2t = scr.tile([P, GC, 2], F16, tag="n2t")
                if c == 0 and KNOB.get("sq_dve", False):
                    # chunk 0 is the critical startup chain: square on DVE
                    # (2x) to skip the ACT round-trip; d-sum via stride-3
                    # views (1x but tiny)
                    sqd = scr.tile([P, GC, 6], F16, tag="sqd")
                    dirs = gtc[:, :, 6:12]
                    nc.vector.tensor_mul(sqd, dirs, dirs)
                    sqv = sqd.rearrange("p g (v d) -> p g v d", d=3)
                    nc.vector.tensor_add(n2a, sqv[:, :, :, 0], sqv[:, :, :, 1])
                    nc.vector.tensor_add(n2t, n2a, sqv[:, :, :, 2])
                else:
                    sqtT = scr.tile([P, GC, 3, 2], F16, tag="sqtT")
                    nc.scalar.activation(
                        sqtT.transpose([0, 1, 3, 2]),
                        gtc[:, :, 6:12].rearrange("p g (v d) -> p g v d", d=3),
                        ACTF.Square,
                    )
                    nc.vector.tensor_add(n2a, sqtT[:, :, 0, :], sqtT[:, :, 1, :])
                    nc.vector.tensor_add(n2t, n2a, sqtT[:, :, 2, :])
                srt = scr.tile([P, GC, 2], F16, tag="srt")
                nc.scalar.activation(srt, n2t, ACTF.Sqrt)  # |r|
                ivt = scr.tile([P, GC, 2], F16, tag="ivt")
                with nc.allow_low_precision(reason="fp16 1/|r|: 2e-2 rel-err budget"):
                    nc.vector.reciprocal(ivt, srt)         # 1/|r|
                cct = scr.tile([P, GC, 3], F16, tag="cct")
                nc.vector.tensor_mul(cct[:, :, 0:2], gtc[:, :, 3:5], ivt)
                tzt = scr.tile([P, GC, 1], F16, tag="tzt")
                nc.vector.tensor_mul(tzt, ivt[:, :, 0:1], ivt[:, :, 1:2])
                nc.vector.tensor_mul(cct[:, :, 2:3], gtc[:, :, 5:6], tzt)

                st0[c] = (gtc, cct)

            def P0b(c):
                r0, GC = CHUNKS[c]
                cs = slice(r0, r0 + GC)
                gtc, cct = st0[c]
                # cross product (raw rx x ry) on GPSIMD
                rxet = xe.tile([P, GC, 5], F16, tag="rxet")
                ryet = xe.tile([P, GC, 5], F16, tag="ryet")
                if c == 0 and KNOB.get("rxy_dve_c0", False):
                    # chunk 0: stage the rotated copies on DVE (4x mode) so
                    # ACT's critical sqrt is not queued behind them; they fill
                    # the DVE gap that waits on sqrt->recip anyway
                    nc.vector.tensor_copy(rxet[:, :, 0:3], gtc[:, :, 6:9])
                    nc.vector.tensor_copy(rxet[:, :, 3:5], gtc[:, :, 6:8])
                    nc.vector.tensor_copy(ryet[:, :, 0:3], gtc[:, :, 9:12])
                    nc.vector.tensor_copy(ryet[:, :, 3:5], gtc[:, :, 9:11])
                else:
                    nc.scalar.copy(rxet[:, :, 0:3], gtc[:, :, 6:9])
                    nc.scalar.copy(rxet[:, :, 3:5], gtc[:, :, 6:8])
                    nc.scalar.copy(ryet[:, :, 0:3], gtc[:, :, 9:12])
                    nc.scalar.copy(ryet[:, :, 3:5], gtc[:, :, 9:11])
                m1t = xe.tile([P, GC, 3], F16, tag="m1t")
                m2t = xe.tile([P, GC, 3], F16, tag="m2t")
                crt = xe.tile([P, GC, 3], F16, tag="crt")
                use_gp = KNOB.get("p0_gp", False) or (
                    c == 0 and KNOB.get("p0_gp_c0", False)
                )
                # cross products touch only raw gt data, so for chunk 0 they
                # can ride Pool's idle pre-d-sum window independent of the
                # scalar chain
                cross_gp = use_gp or (c == 0 and KNOB.get("cross_gp_c0", False))
                def p0_tt(o_, a_, b_, op_, cross=False):
                    if use_gp or (cross and cross_gp):
                        gp_tt(o_, a_, b_, op_)
                    else:
                        nc.vector.tensor_tensor(o_, a_, b_, op=op_)
                p0_tt(m1t, rxet[:, :, 1:4], ryet[:, :, 2:5], ALU.mult,
                      cross=True)
                p0_tt(m2t, rxet[:, :, 2:5], ryet[:, :, 1:4], ALU.mult,
                      cross=True)
                p0_tt(crt, m1t, m2t, ALU.subtract, cross=True)

                # v01 = dirs * c01 ; v2 = cross * cz. The c scalars are
                # pre-expanded over d on ACT so the muls run in DVE 2x mode
                cce = xe.tile([P, GC, 3, 3], F16, tag="cce")
                if c == 0 and KNOB.get("cce_dve_c0", False):
                    # chunk 0: expand on DVE (1x, broadcast input) - it sits
                    # in the DVE bubble that otherwise waits for ACT here
                    nc.vector.tensor_copy(
                        cce, cct.unsqueeze(3).broadcast_to((P, GC, 3, 3))
                    )
                else:
                    nc.scalar.copy(
                        cce, cct.unsqueeze(3).broadcast_to((P, GC, 3, 3))
                    )
                v01t = xe.tile([P, GC, 2, 3], F16, tag="v01t")
                p0_tt(v01t[:, :, 0, :], gtc[:, :, 6:9], cce[:, :, 0], ALU.mult)
                p0_tt(v01t[:, :, 1, :], gtc[:, :, 9:12], cce[:, :, 1], ALU.mult)
                # basis-change: wt/v2t land in basa so S1 dots use 3 basis
                # vectors instead of 4 corners (mul 96->72, dsum 64->48)
                if HW:
                    v2t = basa[:, cs, 2, :]
                else:
                    v2t = xe.tile([P, GC, 3], F16, tag="v2t", name="v2t")
                p0_tt(v2t, crt, cce[:, :, 2], ALU.mult)

                # u combos (DVE 2x fp16 packed, or Pool via knob)
                if HW:
                    wt = basa[:, cs, 0:2, :]
                else:
                    wt = scr.tile([P, GC, 2, 3], F16, tag="wt", name="wt")
                utc = uta[:, cs]
                v2b = v2t.unsqueeze(2).broadcast_to((P, GC, 2, 3))
                wt_gp = KNOB["wtutc_gp"] or (
                    c == 0 and KNOB.get("wt_gp_c0", False))
                utc_gp = KNOB["wtutc_gp"] or (
                    c == 0 and KNOB.get("utc_gp_c0", False))
                if wt_gp or utc_gp:
                    we = gp_tt if wt_gp else (
                        lambda o, a, b, op: nc.vector.tensor_tensor(
                            o, a, b, op=op))
                    ue_ = gp_tt if utc_gp else (
                        lambda o, a, b, op: nc.vector.tensor_tensor(
                            o, a, b, op=op))
                    we(wt[:, :, 0, :], v01t[:, :, 0, :], v01t[:, :, 1, :],
                       ALU.add)
                    we(wt[:, :, 1, :], v01t[:, :, 0, :], v01t[:, :, 1, :],
                       ALU.subtract)
                    ue_(utc[:, :, 0:2, :], wt, v2b, ALU.add)
                    ue_(utc[:, :, 2:4, :], wt, v2b, ALU.subtract)
                else:
                    nc.vector.tensor_add(
                        wt[:, :, 0, :], v01t[:, :, 0, :], v01t[:, :, 1, :]
                    )
                    nc.vector.tensor_sub(
                        wt[:, :, 1, :], v01t[:, :, 0, :], v01t[:, :, 1, :]
                    )
                    nc.vector.tensor_add(utc[:, :, 0:2, :], wt, v2b)
                    nc.vector.tensor_sub(utc[:, :, 2:4, :], wt, v2b)

                # a2: squares (ACT, accum -> SQA slot), d-sum in fp16 2x
                squtT = xe.tile([P, GC, 3, 4], F16, tag="squtT")
                nc.scalar.activation(
                    squtT.transpose([0, 1, 3, 2]), uta[:, cs], ACTF.Square,
                    scale=0.5,  # (u'/2)^2 = u^2
                    accum_out=accT[:, c, SQA : SQA + 1],
                )
                # with t2pair only corners 0:2 (A+, A-) are read downstream
                NA = 2 if KNOB.get("t2pair", False) else 4
                a2s = scr.tile([P, GC, NA], F16, tag="a2s")
                if c == 0 and KNOB.get("a2s_gp_c0", False):
                    gp_tt(a2s, squtT[:, :, 0, 0:NA], squtT[:, :, 1, 0:NA],
                          ALU.add)
                    gp_tt(a2ba[:, cs, 0:NA], a2s, squtT[:, :, 2, 0:NA],
                          ALU.add)
                else:
                    nc.vector.tensor_add(
                        a2s, squtT[:, :, 0, 0:NA], squtT[:, :, 1, 0:NA]
                    )
                    nc.vector.tensor_add(
                        a2ba[:, cs, 0:NA], a2s, squtT[:, :, 2, 0:NA]
                    )

            # ================= phase 1: pairwise chamfer, pipelined ===========
            # stage S1: DMA + dot mul; S2: d-sum (GP) + b2; S3: |g|/eeb/t2b;
            # S4: min-trees + fused sum. Emission interleaves tiles with skew
            # so each engine's in-order queue never waits on a cross-engine
            # producer that was emitted in the same stage.
            st = [dict() for _ in range(ntiles)]

            def S0(t):
                s = st[t]
                r0, Gt = TILES[t]
                sl = slice(r0, r0 + Gt)
                bt = io.tile([P, Gt, 8, 3], F16, tag="pred")
                pst = io.tile([P, Gt, 3], F16, tag="ps")
                nc.sync.dma_start(
                    out=bt, in_=pred_r[:, sl].rearrange("p g (q d) -> p g q d", d=3)
                )
                nc.sync.dma_start(out=pst, in_=ps_r[:, sl])
                s["bt"], s["pst"] = bt, pst

            NI = 3 if HW else 4  # dot-vectors per row: basis 3 or corners 4

            def S1(t):
                s = st[t]
                r0, Gt = TILES[t]
                sl = slice(r0, r0 + Gt)
                bt = s["bt"]
                mt = xe.tile([P, Gt, NI, 8, 3], F16, tag="mt")
                src = basa if HW else uta
                ue = src[:, sl].unsqueeze(3).broadcast_to((P, Gt, NI, 8, 3))
                be = bt.unsqueeze(2).broadcast_to((P, Gt, NI, 8, 3))
                if t == 0 and KNOB.get("mt_gp_t0", False):
                    gp_tt(mt, ue, be, ALU.mult)
                elif t == 1 and KNOB.get("mt_gp_t1_i", 0):
                    # slice of tile-1's mul in Pool's idle head window
                    ni = KNOB["mt_gp_t1_i"]
                    gp_tt(mt[:, :, 0:ni], ue[:, :, 0:ni], be[:, :, 0:ni],
                          ALU.mult)
                    nc.vector.tensor_mul(
                        mt[:, :, ni:NI], ue[:, :, ni:NI], be[:, :, ni:NI]
                    )
                else:
                    nc.vector.tensor_mul(mt, ue, be)
                s["mt"] = mt
                s["mtf"] = mt.rearrange("p g i q d -> p (g i) q d")

            def S2(t):
                s = st[t]
                r0, Gt = TILES[t]
                bt = s["bt"]
                mtf = s["mtf"]
                mt = s["mt"]
                d01 = xe.tile([P, Gt, NI, 8], F16, tag="d01", bufs=KNOB.get("d01_bufs", 2))
                d01f = d01.rearrange("p g i q -> p (g i) q")
                gb = xe.tile([P, Gt, NI, 8], F16, tag="gb")
                mode = KNOB.get("dsum", "pp")
                if mode == "pp":
                    gp_tt(d01f, mtf[:, :, :, 0], mtf[:, :, :, 1], ALU.add)
                    gp_tt(gb.rearrange("p g i q -> p (g i) q"), d01f,
                          mtf[:, :, :, 2], ALU.add)
                elif mode == "pd":   # d01 Pool, gb DVE
                    gp_tt(d01f, mtf[:, :, :, 0], mtf[:, :, :, 1], ALU.add)
                    nc.vector.tensor_add(gb, d01, mt[:, :, :, :, 2])
                elif mode == "dp":   # d01 DVE, gb Pool
                    nc.vector.tensor_add(d01, mt[:, :, :, :, 0], mt[:, :, :, :, 1])
                    gp_tt(gb.rearrange("p g i q -> p (g i) q"), d01f,
                          mtf[:, :, :, 2], ALU.add)
                elif mode == "split":  # d01 Pool; gb: half DVE, half Pool
                    gp_tt(d01f, mtf[:, :, :, 0], mtf[:, :, :, 1], ALU.add)
                    nc.vector.tensor_add(gb[:, :, 0:2], d01[:, :, 0:2],
                                         mt[:, :, 0:2, :, 2])
                    gp_tt(gb[:, :, 2:4], d01[:, :, 2:4], mt[:, :, 2:4, :, 2],
                          ALU.add)
                sqbtT = xe.tile([P, Gt, 3, 8], F16, tag="sqbtT")
                nc.scalar.activation(
                    sqbtT.transpose([0, 1, 3, 2]), bt, ACTF.Square,
                    accum_out=accT[:, t, SQB : SQB + 1],
                )
                b2s = scr.tile([P, Gt, 8], F16, tag="b2s")
                b2b = xe.tile([P, Gt, 8], F16, tag="b2b")
                if (KNOB.get("b2_gp", False) or t < KNOB.get("b2_gp_head", 0)
                        or (KNOB.get("b2_gp_tail", False)
                            and t >= ntiles - KNOB.get("b2_tail_n", 1))):
                    # early tiles: Pool is idle until the first d-sum, so
                    # their b2 sums ride there for free
                    gp_tt(b2s, sqbtT[:, :, 0, :], sqbtT[:, :, 1, :], ALU.add)
                    gp_tt(b2b, b2s, sqbtT[:, :, 2, :], ALU.add)
                else:
                    nc.vector.tensor_add(b2s, sqbtT[:, :, 0, :], sqbtT[:, :, 1, :])
                    nc.vector.tensor_add(b2b, b2s, sqbtT[:, :, 2, :])
                if HW and KNOB.get("exp_gp", False):
                    g4 = xe.tile([P, Gt, 4, 8], F16, tag="g4")
                    wb = gb[:, :, 0:2, :]
                    w2bc = gb[:, :, 2:3, :].broadcast_to((P, Gt, 2, 8))
                    gp_tt(g4[:, :, 0:2, :], wb, w2bc, ALU.add)
                    gp_tt(g4[:, :, 2:4, :], wb, w2bc, ALU.subtract)
                    gb = g4
                if KNOB.get("eebm_s2", 0):
                    # eeb-minus rows on Pool, emitted right behind this
                    # tile's d-sum so the Pool queue adds no latency
                    r = KNOB["eebm_s2"]
                    eeb = xe.tile([P, Gt, 8, 8], F16, tag="eeb")
                    b2bc = b2b.unsqueeze(2).broadcast_to((P, Gt, 4, 8))
                    gp_tt(eeb[:, :, 0:r, :], b2bc[:, :, 0:r], gb[:, :, 0:r],
                          ALU.subtract)
                    s["eeb_pre"] = eeb
                s["gb"], s["b2b"] = gb, b2b

            def S3(t):
                s = st[t]
                r0, Gt = TILES[t]
                gb, b2b = s["gb"], s["b2b"]
                sl = slice(r0, r0 + Gt)
                if HW and not KNOB.get("exp_gp", False):
                    # expand basis dots [t,s,w2] -> 4 corner dots on DVE (2x)
                    g4 = xe.tile([P, Gt, 4, 8], F16, tag="g4")
                    wb = gb[:, :, 0:2, :]
                    w2bc = gb[:, :, 2:3, :].broadcast_to((P, Gt, 2, 8))
                    nc.vector.tensor_add(g4[:, :, 0:2, :], wb, w2bc)
                    nc.vector.tensor_sub(g4[:, :, 2:4, :], wb, w2bc)
                    gb = s["gb"] = g4
                agbT = xe.tile([P, Gt, 8, 4], F16, tag="agbT")
                nc.scalar.activation(agbT.transpose([0, 1, 3, 2]), gb, ACTF.Abs)
                b2bc = b2b.unsqueeze(2).broadcast_to((P, Gt, 4, 8))
                if "eeb_pre" in s:
                    eeb = s["eeb_pre"]
                    embgp = KNOB["eebm_s2"]
                else:
                    eeb = xe.tile([P, Gt, 8, 8], F16, tag="eeb", bufs=KNOB.get("eeb_bufs", 2))
                    embgp = KNOB.get("eebm_gp", 0)  # eeb-minus rows on Pool
                    if embgp > 0:
                        gp_tt(eeb[:, :, 0:embgp, :], b2bc[:, :, 0:embgp],
                              gb[:, :, 0:embgp], ALU.subtract)
                if embgp < 4:
                    nc.vector.tensor_sub(
                        eeb[:, :, embgp:4, :], b2bc[:, :, embgp:4],
                        gb[:, :, embgp:4]
                    )
                if KNOB.get("eebp_gp", False):
                    gp_tt(eeb[:, :, 4:8, :], b2bc, gb, ALU.add)
                else:
                    nc.vector.tensor_add(eeb[:, :, 4:8, :], b2bc, gb)
                if KNOB.get("t2pair", False):
                    # corners pair up: |u0|=|u2|=A+, |u1|=|u3|=A- (v2 is the
                    # cross product, orthogonal to v0/v1), so
                    # t2 = min(A+ - max(|g0|,|g2|), A- - max(|g1|,|g3|)).
                    # M: lane pairs (0,2),(1,3) are exactly the packed halves.
                    Mx = xe.tile([P, Gt, 8, 2], F16, tag="Mx", bufs=KNOB.get("sf_bufs", 2))
                    nc.vector.tensor_tensor(
                        Mx, agbT[:, :, :, 0:2], agbT[:, :, :, 2:4], op=ALU.max
                    )
                    # SF rows 0:8 = e1 pair-mins (S4), rows 8:16 = t2 lanes
                    SF = xe.tile([P, Gt, 16, 2], F16, tag="SF", bufs=KNOB.get("sf_bufs", 2))
                    a2bc2 = a2ba[:, sl, 0:2].unsqueeze(2).broadcast_to(
                        (P, Gt, 8, 2)
                    )
                    nc.vector.tensor_sub(SF[:, :, 8:16, :], a2bc2, Mx)
                    s["SF"] = SF
                    ET = None
                else:
                    a2bcT = a2ba[:, sl].unsqueeze(2).broadcast_to((P, Gt, 8, 4))
                    # e1 and t2 share one tile: their pair-min is ONE op
                    ET = xe.tile([P, Gt, 16, 4], F16, tag="ET")
                    t2gp = KNOB.get("t2_gp", 0)  # 0..8 rows of t2 sub on Pool
                    if t2gp > 0:
                        gp_tt(ET[:, :, 8 : 8 + t2gp, :], a2bcT[:, :, 0:t2gp],
                              agbT[:, :, 0:t2gp], ALU.subtract)
                    if t2gp < 8:
                        nc.vector.tensor_sub(
                            ET[:, :, 8 + t2gp : 16, :], a2bcT[:, :, t2gp:8],
                            agbT[:, :, t2gp:8]
                        )
                if t in dumpts:
                    l1d = dumpts[t][:, :, 16:19]
                else:
                    l1d = xe.tile([P, Gt, 3], F16, tag="l1d", name="l1d")
                if (KNOB.get("l1d_gp", True)
                        or t < KNOB.get("l1d_gp_head", 0)
                        or (KNOB.get("l1d_gp_tail", False)
                            and t >= ntiles - KNOB.get("l1d_tail_n", 1))):
                    gp_tt(l1d, s["pst"], gta[:, sl, 3:6], ALU.subtract)
                else:
                    nc.vector.tensor_sub(l1d, s["pst"], gta[:, sl, 3:6])
                s["eeb"], s["ET"], s["l1d"] = eeb, ET, l1d

            def S4(t):
                s = st[t]
                r0, Gt = TILES[t]
                eeb = s["eeb"]
                if KNOB.get("t2pair", False):
                    SF = s["SF"]
                    E4 = xe.tile([P, Gt, 8, 4], F16, tag="E4", bufs=KNOB.get("sf_bufs", 2))
                    nc.vector.tensor_tensor(
                        E4, eeb[:, :, :, 0:4], eeb[:, :, :, 4:8], op=ALU.min
                    )
                    nc.vector.tensor_tensor(
                        SF[:, :, 0:8, :], E4[:, :, :, 0:2], E4[:, :, :, 2:4],
                        op=ALU.min,
                    )
                    ett = SF
                else:
                    ET = s["ET"]
                    nc.vector.tensor_tensor(
                        ET[:, :, 0:8, :], eeb[:, :, :, 0:4], eeb[:, :, :, 4:8],
                        op=ALU.min,
                    )
                    # alternate tags = double-buffer: the next tile's St write
                    # must not wait on this tile's junk16 ACT read (WAR)
                    ett = scr.tile([P, Gt, 16, 2], F16, tag=f"ett{t % 2}")
                    nc.vector.tensor_tensor(
                        ett, ET[:, :, :, 0:2], ET[:, :, :, 2:4], op=ALU.min
                    )
                if t in dumpts:
                    St = dumpts[t][:, :, 0:16]
                else:
                    St = scr.tile([P, Gt, 16], F16, tag=f"St{t % 2}",
                                  name="St")
                if t == ntiles - 1 and KNOB.get("st_split_tail", False):
                    # split the final min by row-halves: the dump DMA then
                    # trails the later of the e1/t2 branches by a half-op
                    nc.vector.tensor_tensor(
                        St[:, :, 0:8], ett[:, :, 0:8, 0], ett[:, :, 0:8, 1],
                        op=ALU.min,
                    )
                    nc.vector.tensor_tensor(
                        St[:, :, 8:16], ett[:, :, 8:16, 0],
                        ett[:, :, 8:16, 1], op=ALU.min,
                    )
                else:
                    nc.vector.tensor_tensor(
                        St, ett[:, :, :, 0], ett[:, :, :, 1], op=ALU.min
                    )
                if t in dumpts:
                    if t == ntiles - 1:
                        # accT via the idle ACT queue: keeps the SP sequencer
                        # free so the final dump's 565ns DMA-seq slot can
                        # pre-run before St completes
                        eng = (nc.scalar if KNOB.get("acct_act_dma", False)
                               else nc.sync)
                        eng.dma_start(
                            out=out, in_=accT.rearrange("p t x -> p (t x)")
                        )
                    # raw dump; host sums St and |l1d| for this tile
                    nc.sync.dma_start(
                        out=outd[t],
                        in_=dumpts[t].rearrange("p g x -> p (g x)")
                    )
                else:
                    junk16 = scr.tile([P, Gt, 16], F16, tag=f"junk16{t % 2}")
                    nc.scalar.activation(
                        junk16, St,
                        ACTF.Identity,
                        accum_out=accT[:, t, MINSUM : MINSUM + 1],
                    )
                    junk3 = scr.tile([P, Gt, 3], F16, tag="junk3")
                    nc.scalar.activation(
                        junk3, s["l1d"], ACTF.Abs,
                        accum_out=accT[:, t, L1 : L1 + 1]
                    )
                s.clear()

            stages = [S0, S1, S2, S3, S4]
            nstage = len(stages)
            # phase-0 chunk 0 first (gates tiles 0..1); chunk 1 emitted two
            # steps in so its Pool work does not collide with tile-0's d-sum.
            # Within a step, emit S1 first: each stage's cross-engine
            # producers then finished a full step earlier, so no engine's
            # in-order queue blocks.
            # chunk schedule: (emit-step, which) pairs; chunk0 up front
            P0a(0)
            P0b(0)
            if KNOB.get("prefetch_all", False):
                for t in range(ntiles):
                    S0(t)
            scheds = {
                "B": {1: [lambda: P0a(1), lambda: P0b(1)]},
                "C": {0: [lambda: P0a(1)], 1: [lambda: P0b(1)]},
                "D": {0: [lambda: P0a(1), lambda: P0b(1)]},
                "E": {2: [lambda: P0a(1), lambda: P0b(1)]},
                "3w": {1: [lambda: P0a(1)], 2: [lambda: P0b(1), lambda: P0a(2)],
                       3: [lambda: P0b(2)]},
                "3x": {0: [lambda: P0a(1)], 1: [lambda: P0b(1), lambda: P0a(2)],
                       2: [lambda: P0b(2)]},
            }
            # genN: chunk c's two sub-stages at steps offset+2(c-1)(+1)
            nch = len(CHUNKS)
            for off in (0, 1, 2):
                g = {}
                for cc in range(1, nch):
                    base_k = off + 2 * (cc - 1)
                    g.setdefault(base_k, []).append(
                        lambda c_=cc: P0a(c_))
                    g.setdefault(base_k + 1, []).append(
                        lambda c_=cc: P0b(c_))
                scheds[f"gen{off}"] = g
            # genNc: compact - both sub-stages of chunk c at one step
            for off in (0, 1, 2):
                g = {}
                for cc in range(1, nch):
                    g.setdefault(off + (cc - 1), []).extend(
                        [lambda c_=cc: P0a(c_), lambda c_=cc: P0b(c_)])
                scheds[f"gen{off}c"] = g
            sched = scheds[KNOB.get("sched", "B")]
            if nch > 2 and KNOB.get("sched", "B") in ("B", "C", "D", "E"):
                sched = scheds["gen1c"]  # legacy scheds only emit chunk 1
            if True:
                for k in range(ntiles + nstage - 1):
                    for fn_ in sched.get(k, ()):
                        fn_()
                    sorder = (range(nstage - 1, -1, -1) if KNOB["order"] == "rev"
                              else range(nstage))
                    for sidx in sorder:
                        t = k - sidx
                        if 0 <= t < ntiles:
                            if sidx == 0 and KNOB.get("prefetch_all", False):
                                continue
                            stages[sidx](t)

            if not DUMP:
                nc.sync.dma_start(
                    out=out, in_=accT.rearrange("p t x -> p (t x)")
                )

    nc.compile()
    return nc


_CACHE = {}


def _get_nc():
    if "nc" not in _CACHE:
        _CACHE["nc"] = build_nc()
    return _CACHE["nc"]


def combine_partials(outs, dumps=None):
    """outs: list of (P, ntiles*NSLOT) arrays -> (cd_sum, l1_sum) float64."""
    tot_min = 0.0
    tot_sqa = 0.0
    tot_sqb = 0.0
    tot_l1 = 0.0
    for o in outs:
        o = o.astype(np.float64).reshape(P, -1, NSLOT)
        tot_min += o[:, :, MINSUM].sum() + o[:, :, T2SUM].sum()
        tot_sqa += o[:, :, SQA].sum()
        tot_sqb += o[:, :, SQB].sum()
        tot_l1 += o[:, :, L1].sum()
    if dumps is not None:
        for d in dumps:
            d = d.astype(np.float64).reshape(P, -1, 19)
            tot_min += d[:, :, 0:16].sum()
            tot_l1 += np.abs(d[:, :, 16:19]).sum()
    cd_sum = tot_min + 2.0 * tot_sqa + tot_sqb
    return cd_sum, tot_l1


def kernel(pred_pts, pred_size, gt_box):
    from concourse.bass_utils import run_bass_kernel_spmd

    pred_pts = np.asarray(pred_pts, dtype=np.float32)
    pred_size = np.asarray(pred_size, dtype=np.float32)
    gt_box = np.asarray(gt_box, dtype=np.float32)

    N = pred_pts.shape[0]
    assert N == N_TOTAL, f"expected {N_TOTAL} rows, got {N}"
    gt_flat = np.ascontiguousarray(gt_box.reshape(N, 12).astype(np.float16))
    pred = np.ascontiguousarray(pred_pts.reshape(N, 24).astype(np.float16))
    ps = np.ascontiguousarray(pred_size.astype(np.float16))

    in_maps = [
        {
            "gt": gt_flat[i * NC_N : (i + 1) * NC_N],
            "pred": pred[i * NC_N : (i + 1) * NC_N],
            "ps": ps[i * NC_N : (i + 1) * NC_N],
        }
        for i in range(N_CORES)
    ]
    res = run_bass_kernel_spmd(_get_nc(), in_maps, core_ids=list(range(N_CORES)))
    dumps = None
    if KNOB.get("dump_last", False):
        dn = KNOB.get("dump_n", 1)
        ntiles = len(TILE_SIZES)
        dumps = []
        for r in res.results:
            for td in range(ntiles - dn, ntiles):
                dumps.append(r[f"outd{td}"])
    cd_sum, l1_sum = combine_partials([r["out"] for r in res.results], dumps)
    cd = cd_sum / (N * 8)
    l1 = l1_sum / (N * 3)
    return np.array([cd, l1], dtype=np.float32)



# revision 62
# speedup vs baseline: 1.0031x; 1.0031x over previous
"""Trainium2 Bass kernel for AxisLengthNetMetric (chamfer-distance + L1-size metric).

Reference computation (per row n of N = 262144):
  gt_box row -> size (cols 3:6), rx (6:9), ry (9:12)
  rx_hat = rx/|rx|, ry_hat = ry/|ry|, rz = cross(rx_hat, ry_hat)
  corners u_c = sum_k sign[c,k] * 0.5*size[k] * axis_k   (8 corners, +-pairs)
  chamfer(corners, pred_pts[n]): d[p,q] = |a_p - b_q|^2, dist1 = min_q, dist2 = min_p
  out[0] = mean over (N,8) of dist1+dist2 ; out[1] = mean |size - pred_size|

Kernel strategy (v6):
- data parallel over 8 cores; per core 32768 rows as 128 partitions x 256.
- 4 distinct corners up to sign (u' = 2u prescaled, g' = u'.b):
    dist1 sums: min_q(b2 -+ g'), dist2: b2_q + min_i(a2_i - |g'|);
    a2/b2/min contributions accumulated via ACT accum_out, host-combined.
- t2pair: rz = rx_hat x ry_hat is orthogonal to both axes, so corner
  norms take only TWO values (|u0|=|u2|=A+, |u1|=|u3|=A-) and
  dist2 = b2 + min(A+ - max(|g0|,|g2|), A- - max(|g1|,|g3|)); the max
  runs on packed lane pairs, cutting 16 DVE elems/row off the t2 path.
- fp16 end to end (inputs DMA'd as fp16): every TensorTensor with all
  operands 2-byte and packed-innermost runs in DVE 2x mode (0.52 ns/elem).
- dot products as ONE DVE mul in [i,q,d] layout (d innermost => both
  broadcast operands stay packed => 2x), then the d-sum (the only work the
  gpsimd engine gets) as two flattened-3D adds on Pool; everything else
  rides DVE (2x) or ACT (abs/squares/copies/accumulations, transposed
  writes for free axis swaps).
- walrus constraints honored: no ScalarTensorTensor on Pool, no gpsimd
  min, no tensor_tensor_reduce; ScalarTensorTensor APs <= 3D.
- phase-1 software-pipelined (5-stage skew) over TILE_SIZES rows: the
  small head tile fills the tile-0 d-sum latency bubble, the small tail
  tile shortens the drain chain. phase-0 in 2 chunks [88,168] aligned to
  the tile-0/1 boundary; chunk-0's squares / rx,ry staging copies / cce
  expansion run on DVE (sq_dve, rxy_dve_c0, cce_dve_c0) to shorten the
  DMA->uta startup chain and fill DVE bubbles there; tile-0's b2 sums and
  l1d and chunk-0's cross products (which need only raw gt data)
  ride the Pool engine's idle pre-d-sum head window; the last tile's b2
  sums ride Pool's idle post-d-sum tail window (b2_gp_tail); chunk-0's
  corner combos + a2 sums (utc/a2s_gp_c0) fill the rest of the head
  window, leaving Pool saturated back-to-back from 12.7us to 33.8us.
- dump_last/dump_n: the last 3 tiles DMA raw St/l1d values (host sums
  them), removing their ACT accumulates from the drain path; the accT DMA
  is issued ahead of the final dump so the two DMA pipelines overlap
  instead of serializing on the SP queue; the last tile's final St min is
  split by row-halves so the dump trails the later branch by a half-op.
- engine busy: DVE ~50.7us, Pool ~41us, ACT ~28us; TimelineSim
  ~58.3us/core (v5: 65.3us, v4: 81.9us).
"""

import numpy as np

import concourse.bacc as bacc
import concourse.bass as bass  # noqa: F401
import concourse.tile as tile
from concourse import mybir

F32 = mybir.dt.float32
F16 = mybir.dt.float16
ALU = mybir.AluOpType
ACTF = mybir.ActivationFunctionType
AX = mybir.AxisListType

P = 128
N_CORES = 8
N_TOTAL = 262144
NC_N = N_TOTAL // N_CORES  # 32768 rows per core
G_PROD = 64                # rows per partition per heavy tile
# engine assignment knobs: True = put op on gpsimd (Pool)
KNOB = {
    "wtutc_gp": False, "xe_bufs": 2, "order": "rev", "p0k": 1, "nchunk": 2,
    "seq": False, "dsum": "pp", "p0_gp": False, "l1d_gp": False,
    "eebp_gp": False, "b2_gp": False, "t2b_gp": False,
    "t2pair": True, "dump_last": True, "rxy_dve_c0": True,
    "sq_dve": True, "chunks": [88, 168], "sched": "gen0c",
    "cce_dve_c0": True, "b2_gp_head": 1, "l1d_gp_head": 1,
    "cross_gp_c0": True, "b2_gp_tail": True, "dump_n": 3,
    "st_split_tail": True, "utc_gp_c0": True, "a2s_gp_c0": True,
}

# accT slots per tile
MINSUM, SQA, SQB, L1, T2SUM = 0, 1, 2, 3, 4
NSLOT = 5


TILE_SIZES = [28, 60, 72, 72, 24]  # shrinking tail shortens the drain chain


def build_nc(nc_n=NC_N, G=G_PROD, tile_sizes=None):
    GA = nc_n // P             # all rows per partition
    if tile_sizes is None:
        tile_sizes = TILE_SIZES if GA == 256 else [G] * (GA // G)
    assert sum(tile_sizes) == GA
    TILES = []
    r = 0
    for gsz in tile_sizes:
        TILES.append((r, gsz))
        r += gsz
    ntiles = len(TILES)

    nc = bacc.Bacc("TRN2", target_bir_lowering=False, debug=False)

    # gt split host-side: gtd = dirs (cols 6:12), gts = sizes (cols 3:6).
    # cols 0:3 were never read, and the startup-critical chunk-0 DMA now
    # moves half the bytes
    gtd = nc.dram_tensor("gtd", [nc_n, 6], F16, kind="ExternalInput").ap()
    gts = nc.dram_tensor("gts", [nc_n, 3], F16, kind="ExternalInput").ap()
    pred = nc.dram_tensor("pred", [nc_n, 24], F16, kind="ExternalInput").ap()
    ps = nc.dram_tensor("ps", [nc_n, 3], F16, kind="ExternalInput").ap()
    out = nc.dram_tensor("out", [P, ntiles * NSLOT], F32, kind="ExternalOutput").ap()
    # last tile dumps raw St/l1d (host sums them): the final ACT accum +
    # its sem hop come off the drain critical path
    DUMP = KNOB.get("dump_last", False)
    DUMP_N = KNOB.get("dump_n", 1) if DUMP else 0
    outd = {}
    for td in range(ntiles - DUMP_N, ntiles):
        outd[td] = nc.dram_tensor(
            f"outd{td}", [P, TILES[td][1] * 19], F16, kind="ExternalOutput"
        ).ap()

    gtd_r = gtd.rearrange("(p g) f -> p g f", p=P)
    gts_r = gts.rearrange("(p g) f -> p g f", p=P)
    pred_r = pred.rearrange("(p g) f -> p g f", p=P)
    ps_r = ps.rearrange("(p g) f -> p g f", p=P)

    def gp_tt(out_, a, b, op):
        # plain TensorTensor on gpsimd (walrus rejects TensorScalarPtr on Pool)
        nc.gpsimd.tensor_tensor(out_, a, b, op=op)

    with tile.TileContext(nc) as tc:
        with (
            tc.tile_pool(name="per", bufs=1) as per,   # persistent / phase-0
            tc.tile_pool(name="io", bufs=KNOB.get("io_bufs", 3)) as io,
            tc.tile_pool(name="scr", bufs=KNOB.get("scr_bufs", 2)) as scr,
            tc.tile_pool(name="xe", bufs=KNOB["xe_bufs"]) as xe,
        ):
            HW = KNOB.get("halfw", False)  # dots vs basis {v0+v1, v0-v1, v2}
            accT = per.tile([P, ntiles, NSLOT], F32)
            # SQA only fills NCHUNK of the ntiles slots - zero the rest
            nc.gpsimd.memset(accT, 0.0)

            # warm the ACT function tables before any data dependency
            warm = per.tile([P, 2], F32)
            nc.vector.memset(warm, 1.0)
            for fn in (ACTF.Sqrt, ACTF.Square, ACTF.Abs, ACTF.Identity):
                nc.scalar.activation(warm[:, 0:1], warm[:, 1:2], fn)

            # ================= phase 0: corner basis (pipelined stage) ========
            gtda = per.tile([P, GA, 6], F16)
            gtsa = per.tile([P, GA, 3], F16)
            uta = per.tile([P, GA, 4, 3], F16)
            a2ba = per.tile([P, GA, 4], F16)
            basa = None  # [p, m, v2] basis per row when halfw
            if HW:
                basa = per.tile([P, GA, 3, 3], F16, name="basa")
            dumpts = {}
            for td in outd:
                dumpts[td] = per.tile(
                    [P, TILES[td][1], 19], F16, name=f"dumpt{td}"
                )
            CH = KNOB.get("chunks", "half")
            if isinstance(CH, (list, tuple)):
                CHUNKS = []
                r0c = 0
                for gc in CH:
                    CHUNKS.append((r0c, gc))
                    r0c += gc
                assert r0c == GA
            elif CH == "half":
                CHUNKS = [(0, GA // 2), (GA // 2, GA // 2)]
            elif CH == "64rest":
                CHUNKS = [(0, 64), (64, GA - 64)]
            elif CH == "3way":
                CHUNKS = [(0, 64), (64, 96), (160, GA - 160)]
            st0 = {}

            def P0a(c):
                r0, GC = CHUNKS[c]
                cs = slice(r0, r0 + GC)
                gtc = gtda[:, cs]
                nc.sync.dma_start(out=gtc, in_=gtd_r[:, cs])
                nc.sync.dma_start(out=gtsa[:, cs], in_=gts_r[:, cs])

                n2a = scr.tile([P, GC, 2], F16, tag="n2a")
                n2t = scr.tile([P, GC, 2], F16, tag="n2t")
                if c == 0 and KNOB.get("sq_dve", False):
                    # chunk 0 is the critical startup chain: square on DVE
                    # (2x) to skip the ACT round-trip; d-sum via stride-3
                    # views (1x but tiny)
                    sqd = scr.tile([P, GC, 6], F16, tag="sqd")
                    dirs = gtc
                    nc.vector.tensor_mul(sqd, dirs, dirs)
                    sqv = sqd.rearrange("p g (v d) -> p g v d", d=3)
                    nc.vector.tensor_add(n2a, sqv[:, :, :, 0], sqv[:, :, :, 1])
                    nc.vector.tensor_add(n2t, n2a, sqv[:, :, :, 2])
                else:
                    sqtT = scr.tile([P, GC, 3, 2], F16, tag="sqtT")
                    nc.scalar.activation(
                        sqtT.transpose([0, 1, 3, 2]),
                        gtc.rearrange("p g (v d) -> p g v d", d=3),
                        ACTF.Square,
                    )
                    nc.vector.tensor_add(n2a, sqtT[:, :, 0, :], sqtT[:, :, 1, :])
                    nc.vector.tensor_add(n2t, n2a, sqtT[:, :, 2, :])
                srt = scr.tile([P, GC, 2], F16, tag="srt")
                nc.scalar.activation(srt, n2t, ACTF.Sqrt)  # |r|
                ivt = scr.tile([P, GC, 2], F16, tag="ivt")
                with nc.allow_low_precision(reason="fp16 1/|r|: 2e-2 rel-err budget"):
                    nc.vector.reciprocal(ivt, srt)         # 1/|r|
                cct = scr.tile([P, GC, 3], F16, tag="cct")
                nc.vector.tensor_mul(cct[:, :, 0:2], gtsa[:, cs, 0:2], ivt)
                tzt = scr.tile([P, GC, 1], F16, tag="tzt")
                nc.vector.tensor_mul(tzt, ivt[:, :, 0:1], ivt[:, :, 1:2])
                nc.vector.tensor_mul(cct[:, :, 2:3], gtsa[:, cs, 2:3], tzt)

                st0[c] = (gtc, cct)

            def P0b(c):
                r0, GC = CHUNKS[c]
                cs = slice(r0, r0 + GC)
                gtc, cct = st0[c]
                # cross product (raw rx x ry) on GPSIMD
                rxet = xe.tile([P, GC, 5], F16, tag="rxet")
                ryet = xe.tile([P, GC, 5], F16, tag="ryet")
                if c == 0 and KNOB.get("rxy_dve_c0", False):
                    # chunk 0: stage the rotated copies on DVE (4x mode) so
                    # ACT's critical sqrt is not queued behind them; they fill
                    # the DVE gap that waits on sqrt->recip anyway
                    nc.vector.tensor_copy(rxet[:, :, 0:3], gtc[:, :, 0:3])
                    nc.vector.tensor_copy(rxet[:, :, 3:5], gtc[:, :, 0:2])
                    nc.vector.tensor_copy(ryet[:, :, 0:3], gtc[:, :, 3:6])
                    nc.vector.tensor_copy(ryet[:, :, 3:5], gtc[:, :, 3:5])
                else:
                    nc.scalar.copy(rxet[:, :, 0:3], gtc[:, :, 0:3])
                    nc.scalar.copy(rxet[:, :, 3:5], gtc[:, :, 0:2])
                    nc.scalar.copy(ryet[:, :, 0:3], gtc[:, :, 3:6])
                    nc.scalar.copy(ryet[:, :, 3:5], gtc[:, :, 3:5])
                m1t = xe.tile([P, GC, 3], F16, tag="m1t")
                m2t = xe.tile([P, GC, 3], F16, tag="m2t")
                crt = xe.tile([P, GC, 3], F16, tag="crt")
                use_gp = KNOB.get("p0_gp", False) or (
                    c == 0 and KNOB.get("p0_gp_c0", False)
                )
                # cross products touch only raw gt data, so for chunk 0 they
                # can ride Pool's idle pre-d-sum window independent of the
                # scalar chain
                cross_gp = use_gp or (c == 0 and KNOB.get("cross_gp_c0", False))
                def p0_tt(o_, a_, b_, op_, cross=False):
                    if use_gp or (cross and cross_gp):
                        gp_tt(o_, a_, b_, op_)
                    else:
                        nc.vector.tensor_tensor(o_, a_, b_, op=op_)
                p0_tt(m1t, rxet[:, :, 1:4], ryet[:, :, 2:5], ALU.mult,
                      cross=True)
                p0_tt(m2t, rxet[:, :, 2:5], ryet[:, :, 1:4], ALU.mult,
                      cross=True)
                p0_tt(crt, m1t, m2t, ALU.subtract, cross=True)

                # v01 = dirs * c01 ; v2 = cross * cz. The c scalars are
                # pre-expanded over d on ACT so the muls run in DVE 2x mode
                cce = xe.tile([P, GC, 3, 3], F16, tag="cce")
                if c == 0 and KNOB.get("cce_dve_c0", False):
                    # chunk 0: expand on DVE (1x, broadcast input) - it sits
                    # in the DVE bubble that otherwise waits for ACT here
                    nc.vector.tensor_copy(
                        cce, cct.unsqueeze(3).broadcast_to((P, GC, 3, 3))
                    )
                else:
                    nc.scalar.copy(
                        cce, cct.unsqueeze(3).broadcast_to((P, GC, 3, 3))
                    )
                v01t = xe.tile([P, GC, 2, 3], F16, tag="v01t")
                p0_tt(v01t[:, :, 0, :], gtc[:, :, 0:3], cce[:, :, 0], ALU.mult)
                p0_tt(v01t[:, :, 1, :], gtc[:, :, 3:6], cce[:, :, 1], ALU.mult)
                # basis-change: wt/v2t land in basa so S1 dots use 3 basis
                # vectors instead of 4 corners (mul 96->72, dsum 64->48)
                if HW:
                    v2t = basa[:, cs, 2, :]
                else:
                    v2t = xe.tile([P, GC, 3], F16, tag="v2t", name="v2t")
                p0_tt(v2t, crt, cce[:, :, 2], ALU.mult)

                # u combos (DVE 2x fp16 packed, or Pool via knob)
                if HW:
                    wt = basa[:, cs, 0:2, :]
                else:
                    wt = scr.tile([P, GC, 2, 3], F16, tag="wt", name="wt")
                utc = uta[:, cs]
                v2b = v2t.unsqueeze(2).broadcast_to((P, GC, 2, 3))
                wt_gp = KNOB["wtutc_gp"] or (
                    c == 0 and KNOB.get("wt_gp_c0", False))
                utc_gp = KNOB["wtutc_gp"] or (
                    c == 0 and KNOB.get("utc_gp_c0", False))
                if wt_gp or utc_gp:
                    we = gp_tt if wt_gp else (
                        lambda o, a, b, op: nc.vector.tensor_tensor(
                            o, a, b, op=op))
                    ue_ = gp_tt if utc_gp else (
                        lambda o, a, b, op: nc.vector.tensor_tensor(
                            o, a, b, op=op))
                    we(wt[:, :, 0, :], v01t[:, :, 0, :], v01t[:, :, 1, :],
                       ALU.add)
                    we(wt[:, :, 1, :], v01t[:, :, 0, :], v01t[:, :, 1, :],
                       ALU.subtract)
                    ue_(utc[:, :, 0:2, :], wt, v2b, ALU.add)
                    ue_(utc[:, :, 2:4, :], wt, v2b, ALU.subtract)
                else:
                    nc.vector.tensor_add(
                        wt[:, :, 0, :], v01t[:, :, 0, :], v01t[:, :, 1, :]
                    )
                    nc.vector.tensor_sub(
                        wt[:, :, 1, :], v01t[:, :, 0, :], v01t[:, :, 1, :]
                    )
                    nc.vector.tensor_add(utc[:, :, 0:2, :], wt, v2b)
                    nc.vector.tensor_sub(utc[:, :, 2:4, :], wt, v2b)

                # a2: squares (ACT, accum -> SQA slot), d-sum in fp16 2x
                squtT = xe.tile([P, GC, 3, 4], F16, tag="squtT")
                nc.scalar.activation(
                    squtT.transpose([0, 1, 3, 2]), uta[:, cs], ACTF.Square,
                    scale=0.5,  # (u'/2)^2 = u^2
                    accum_out=accT[:, c, SQA : SQA + 1],
                )
                # with t2pair only corners 0:2 (A+, A-) are read downstream
                NA = 2 if KNOB.get("t2pair", False) else 4
                a2s = scr.tile([P, GC, NA], F16, tag="a2s")
                if c == 0 and KNOB.get("a2s_gp_c0", False):
                    gp_tt(a2s, squtT[:, :, 0, 0:NA], squtT[:, :, 1, 0:NA],
                          ALU.add)
                    gp_tt(a2ba[:, cs, 0:NA], a2s, squtT[:, :, 2, 0:NA],
                          ALU.add)
                else:
                    nc.vector.tensor_add(
                        a2s, squtT[:, :, 0, 0:NA], squtT[:, :, 1, 0:NA]
                    )
                    nc.vector.tensor_add(
                        a2ba[:, cs, 0:NA], a2s, squtT[:, :, 2, 0:NA]
                    )

            # ================= phase 1: pairwise chamfer, pipelined ===========
            # stage S1: DMA + dot mul; S2: d-sum (GP) + b2; S3: |g|/eeb/t2b;
            # S4: min-trees + fused sum. Emission interleaves tiles with skew
            # so each engine's in-order queue never waits on a cross-engine
            # producer that was emitted in the same stage.
            st = [dict() for _ in range(ntiles)]

            def S0(t):
                s = st[t]
                r0, Gt = TILES[t]
                sl = slice(r0, r0 + Gt)
                bt = io.tile([P, Gt, 8, 3], F16, tag="pred")
                pst = io.tile([P, Gt, 3], F16, tag="ps")
                nc.sync.dma_start(
                    out=bt, in_=pred_r[:, sl].rearrange("p g (q d) -> p g q d", d=3)
                )
                nc.sync.dma_start(out=pst, in_=ps_r[:, sl])
                s["bt"], s["pst"] = bt, pst

            NI = 3 if HW else 4  # dot-vectors per row: basis 3 or corners 4

            def S1(t):
                s = st[t]
                r0, Gt = TILES[t]
                sl = slice(r0, r0 + Gt)
                bt = s["bt"]
                mt = xe.tile([P, Gt, NI, 8, 3], F16, tag="mt")
                src = basa if HW else uta
                ue = src[:, sl].unsqueeze(3).broadcast_to((P, Gt, NI, 8, 3))
                be = bt.unsqueeze(2).broadcast_to((P, Gt, NI, 8, 3))
                if t == 0 and KNOB.get("mt_gp_t0", False):
                    gp_tt(mt, ue, be, ALU.mult)
                elif t == 1 and KNOB.get("mt_gp_t1_i", 0):
                    # slice of tile-1's mul in Pool's idle head window
                    ni = KNOB["mt_gp_t1_i"]
                    gp_tt(mt[:, :, 0:ni], ue[:, :, 0:ni], be[:, :, 0:ni],
                          ALU.mult)
                    nc.vector.tensor_mul(
                        mt[:, :, ni:NI], ue[:, :, ni:NI], be[:, :, ni:NI]
                    )
                else:
                    nc.vector.tensor_mul(mt, ue, be)
                s["mt"] = mt
                s["mtf"] = mt.rearrange("p g i q d -> p (g i) q d")

            def S2(t):
                s = st[t]
                r0, Gt = TILES[t]
                bt = s["bt"]
                mtf = s["mtf"]
                mt = s["mt"]
                d01 = xe.tile([P, Gt, NI, 8], F16, tag="d01", bufs=KNOB.get("d01_bufs", 2))
                d01f = d01.rearrange("p g i q -> p (g i) q")
                gb = xe.tile([P, Gt, NI, 8], F16, tag="gb")
                mode = KNOB.get("dsum", "pp")
                if mode == "pp":
                    gp_tt(d01f, mtf[:, :, :, 0], mtf[:, :, :, 1], ALU.add)
                    gp_tt(gb.rearrange("p g i q -> p (g i) q"), d01f,
                          mtf[:, :, :, 2], ALU.add)
                elif mode == "pd":   # d01 Pool, gb DVE
                    gp_tt(d01f, mtf[:, :, :, 0], mtf[:, :, :, 1], ALU.add)
                    nc.vector.tensor_add(gb, d01, mt[:, :, :, :, 2])
                elif mode == "dp":   # d01 DVE, gb Pool
                    nc.vector.tensor_add(d01, mt[:, :, :, :, 0], mt[:, :, :, :, 1])
                    gp_tt(gb.rearrange("p g i q -> p (g i) q"), d01f,
                          mtf[:, :, :, 2], ALU.add)
                elif mode == "split":  # d01 Pool; gb: half DVE, half Pool
                    gp_tt(d01f, mtf[:, :, :, 0], mtf[:, :, :, 1], ALU.add)
                    nc.vector.tensor_add(gb[:, :, 0:2], d01[:, :, 0:2],
                                         mt[:, :, 0:2, :, 2])
                    gp_tt(gb[:, :, 2:4], d01[:, :, 2:4], mt[:, :, 2:4, :, 2],
                          ALU.add)
                sqbtT = xe.tile([P, Gt, 3, 8], F16, tag="sqbtT")
                nc.scalar.activation(
                    sqbtT.transpose([0, 1, 3, 2]), bt, ACTF.Square,
                    accum_out=accT[:, t, SQB : SQB + 1],
                )
                b2s = scr.tile([P, Gt, 8], F16, tag="b2s")
                b2b = xe.tile([P, Gt, 8], F16, tag="b2b")
                if (KNOB.get("b2_gp", False) or t < KNOB.get("b2_gp_head", 0)
                        or (KNOB.get("b2_gp_tail", False)
                            and t >= ntiles - KNOB.get("b2_tail_n", 1))):
                    # early tiles: Pool is idle until the first d-sum, so
                    # their b2 sums ride there for free
                    gp_tt(b2s, sqbtT[:, :, 0, :], sqbtT[:, :, 1, :], ALU.add)
                    gp_tt(b2b, b2s, sqbtT[:, :, 2, :], ALU.add)
                else:
                    nc.vector.tensor_add(b2s, sqbtT[:, :, 0, :], sqbtT[:, :, 1, :])
                    nc.vector.tensor_add(b2b, b2s, sqbtT[:, :, 2, :])
                if HW and KNOB.get("exp_gp", False):
                    g4 = xe.tile([P, Gt, 4, 8], F16, tag="g4")
                    wb = gb[:, :, 0:2, :]
                    w2bc = gb[:, :, 2:3, :].broadcast_to((P, Gt, 2, 8))
                    gp_tt(g4[:, :, 0:2, :], wb, w2bc, ALU.add)
                    gp_tt(g4[:, :, 2:4, :], wb, w2bc, ALU.subtract)
                    gb = g4
                if KNOB.get("eebm_s2", 0):
                    # eeb-minus rows on Pool, emitted right behind this
                    # tile's d-sum so the Pool queue adds no latency
                    r = KNOB["eebm_s2"]
                    eeb = xe.tile([P, Gt, 8, 8], F16, tag="eeb")
                    b2bc = b2b.unsqueeze(2).broadcast_to((P, Gt, 4, 8))
                    gp_tt(eeb[:, :, 0:r, :], b2bc[:, :, 0:r], gb[:, :, 0:r],
                          ALU.subtract)
                    s["eeb_pre"] = eeb
                s["gb"], s["b2b"] = gb, b2b

            def S3(t):
                s = st[t]
                r0, Gt = TILES[t]
                gb, b2b = s["gb"], s["b2b"]
                sl = slice(r0, r0 + Gt)
                if HW and not KNOB.get("exp_gp", False):
                    # expand basis dots [t,s,w2] -> 4 corner dots on DVE (2x)
                    g4 = xe.tile([P, Gt, 4, 8], F16, tag="g4")
                    wb = gb[:, :, 0:2, :]
                    w2bc = gb[:, :, 2:3, :].broadcast_to((P, Gt, 2, 8))
                    nc.vector.tensor_add(g4[:, :, 0:2, :], wb, w2bc)
                    nc.vector.tensor_sub(g4[:, :, 2:4, :], wb, w2bc)
                    gb = s["gb"] = g4
                agbT = xe.tile([P, Gt, 8, 4], F16, tag="agbT")
                nc.scalar.activation(agbT.transpose([0, 1, 3, 2]), gb, ACTF.Abs)
                b2bc = b2b.unsqueeze(2).broadcast_to((P, Gt, 4, 8))
                if "eeb_pre" in s:
                    eeb = s["eeb_pre"]
                    embgp = KNOB["eebm_s2"]
                else:
                    eeb = xe.tile([P, Gt, 8, 8], F16, tag="eeb", bufs=KNOB.get("eeb_bufs", 2))
                    embgp = KNOB.get("eebm_gp", 0)  # eeb-minus rows on Pool
                    if embgp > 0:
                        gp_tt(eeb[:, :, 0:embgp, :], b2bc[:, :, 0:embgp],
                              gb[:, :, 0:embgp], ALU.subtract)
                if embgp < 4:
                    nc.vector.tensor_sub(
                        eeb[:, :, embgp:4, :], b2bc[:, :, embgp:4],
                        gb[:, :, embgp:4]
                    )
                if KNOB.get("eebp_gp", False):
                    gp_tt(eeb[:, :, 4:8, :], b2bc, gb, ALU.add)
                else:
                    nc.vector.tensor_add(eeb[:, :, 4:8, :], b2bc, gb)
                if KNOB.get("t2pair", False):
                    # corners pair up: |u0|=|u2|=A+, |u1|=|u3|=A- (v2 is the
                    # cross product, orthogonal to v0/v1), so
                    # t2 = min(A+ - max(|g0|,|g2|), A- - max(|g1|,|g3|)).
                    # M: lane pairs (0,2),(1,3) are exactly the packed halves.
                    Mx = xe.tile([P, Gt, 8, 2], F16, tag="Mx", bufs=KNOB.get("sf_bufs", 2))
                    nc.vector.tensor_tensor(
                        Mx, agbT[:, :, :, 0:2], agbT[:, :, :, 2:4], op=ALU.max
                    )
                    # SF rows 0:8 = e1 pair-mins (S4), rows 8:16 = t2 lanes
                    SF = xe.tile([P, Gt, 16, 2], F16, tag="SF", bufs=KNOB.get("sf_bufs", 2))
                    a2bc2 = a2ba[:, sl, 0:2].unsqueeze(2).broadcast_to(
                        (P, Gt, 8, 2)
                    )
                    nc.vector.tensor_sub(SF[:, :, 8:16, :], a2bc2, Mx)
                    s["SF"] = SF
                    ET = None
                else:
                    a2bcT = a2ba[:, sl].unsqueeze(2).broadcast_to((P, Gt, 8, 4))
                    # e1 and t2 share one tile: their pair-min is ONE op
                    ET = xe.tile([P, Gt, 16, 4], F16, tag="ET")
                    t2gp = KNOB.get("t2_gp", 0)  # 0..8 rows of t2 sub on Pool
                    if t2gp > 0:
                        gp_tt(ET[:, :, 8 : 8 + t2gp, :], a2bcT[:, :, 0:t2gp],
                              agbT[:, :, 0:t2gp], ALU.subtract)
                    if t2gp < 8:
                        nc.vector.tensor_sub(
                            ET[:, :, 8 + t2gp : 16, :], a2bcT[:, :, t2gp:8],
                            agbT[:, :, t2gp:8]
                        )
                if t in dumpts:
                    l1d = dumpts[t][:, :, 16:19]
                else:
                    l1d = xe.tile([P, Gt, 3], F16, tag="l1d", name="l1d")
                if (KNOB.get("l1d_gp", True)
                        or t < KNOB.get("l1d_gp_head", 0)
                        or (KNOB.get("l1d_gp_tail", False)
                            and t >= ntiles - KNOB.get("l1d_tail_n", 1))):
                    gp_tt(l1d, s["pst"], gtsa[:, sl, :], ALU.subtract)
                else:
                    nc.vector.tensor_sub(l1d, s["pst"], gtsa[:, sl, :])
                s["eeb"], s["ET"], s["l1d"] = eeb, ET, l1d

            def S4(t):
                s = st[t]
                r0, Gt = TILES[t]
                eeb = s["eeb"]
                if KNOB.get("t2pair", False):
                    SF = s["SF"]
                    E4 = xe.tile([P, Gt, 8, 4], F16, tag="E4", bufs=KNOB.get("sf_bufs", 2))
                    nc.vector.tensor_tensor(
                        E4, eeb[:, :, :, 0:4], eeb[:, :, :, 4:8], op=ALU.min
                    )
                    nc.vector.tensor_tensor(
                        SF[:, :, 0:8, :], E4[:, :, :, 0:2], E4[:, :, :, 2:4],
                        op=ALU.min,
                    )
                    ett = SF
                else:
                    ET = s["ET"]
                    nc.vector.tensor_tensor(
                        ET[:, :, 0:8, :], eeb[:, :, :, 0:4], eeb[:, :, :, 4:8],
                        op=ALU.min,
                    )
                    # alternate tags = double-buffer: the next tile's St write
                    # must not wait on this tile's junk16 ACT read (WAR)
                    ett = scr.tile([P, Gt, 16, 2], F16, tag=f"ett{t % 2}")
                    nc.vector.tensor_tensor(
                        ett, ET[:, :, :, 0:2], ET[:, :, :, 2:4], op=ALU.min
                    )
                if t in dumpts:
                    St = dumpts[t][:, :, 0:16]
                else:
                    St = scr.tile([P, Gt, 16], F16, tag=f"St{t % 2}",
                                  name="St")
                if t == ntiles - 1 and KNOB.get("st_split_tail", False):
                    # split the final min by row-halves: the dump DMA then
                    # trails the later of the e1/t2 branches by a half-op
                    nc.vector.tensor_tensor(
                        St[:, :, 0:8], ett[:, :, 0:8, 0], ett[:, :, 0:8, 1],
                        op=ALU.min,
                    )
                    nc.vector.tensor_tensor(
                        St[:, :, 8:16], ett[:, :, 8:16, 0],
                        ett[:, :, 8:16, 1], op=ALU.min,
                    )
                else:
                    nc.vector.tensor_tensor(
                        St, ett[:, :, :, 0], ett[:, :, :, 1], op=ALU.min
                    )
                if t in dumpts:
                    if t == ntiles - 1:
                        # accT via the idle ACT queue: keeps the SP sequencer
                        # free so the final dump's 565ns DMA-seq slot can
                        # pre-run before St completes
                        eng = (nc.scalar if KNOB.get("acct_act_dma", False)
                               else nc.sync)
                        eng.dma_start(
                            out=out, in_=accT.rearrange("p t x -> p (t x)")
                        )
                    # raw dump; host sums St and |l1d| for this tile
                    nc.sync.dma_start(
                        out=outd[t],
                        in_=dumpts[t].rearrange("p g x -> p (g x)")
                    )
                else:
                    junk16 = scr.tile([P, Gt, 16], F16, tag=f"junk16{t % 2}")
                    nc.scalar.activation(
                        junk16, St,
                        ACTF.Identity,
                        accum_out=accT[:, t, MINSUM : MINSUM + 1],
                    )
                    junk3 = scr.tile([P, Gt, 3], F16, tag="junk3")
                    nc.scalar.activation(
                        junk3, s["l1d"], ACTF.Abs,
                        accum_out=accT[:, t, L1 : L1 + 1]
                    )
                s.clear()

            stages = [S0, S1, S2, S3, S4]
            nstage = len(stages)
            # phase-0 chunk 0 first (gates tiles 0..1); chunk 1 emitted two
            # steps in so its Pool work does not collide with tile-0's d-sum.
            # Within a step, emit S1 first: each stage's cross-engine
            # producers then finished a full step earlier, so no engine's
            # in-order queue blocks.
            # chunk schedule: (emit-step, which) pairs; chunk0 up front
            P0a(0)
            P0b(0)
            if KNOB.get("prefetch_all", False):
                for t in range(ntiles):
                    S0(t)
            scheds = {
                "B": {1: [lambda: P0a(1), lambda: P0b(1)]},
                "C": {0: [lambda: P0a(1)], 1: [lambda: P0b(1)]},
                "D": {0: [lambda: P0a(1), lambda: P0b(1)]},
                "E": {2: [lambda: P0a(1), lambda: P0b(1)]},
                "3w": {1: [lambda: P0a(1)], 2: [lambda: P0b(1), lambda: P0a(2)],
                       3: [lambda: P0b(2)]},
                "3x": {0: [lambda: P0a(1)], 1: [lambda: P0b(1), lambda: P0a(2)],
                       2: [lambda: P0b(2)]},
            }
            # genN: chunk c's two sub-stages at steps offset+2(c-1)(+1)
            nch = len(CHUNKS)
            for off in (0, 1, 2):
                g = {}
                for cc in range(1, nch):
                    base_k = off + 2 * (cc - 1)
                    g.setdefault(base_k, []).append(
                        lambda c_=cc: P0a(c_))
                    g.setdefault(base_k + 1, []).append(
                        lambda c_=cc: P0b(c_))
                scheds[f"gen{off}"] = g
            # genNc: compact - both sub-stages of chunk c at one step
            for off in (0, 1, 2):
                g = {}
                for cc in range(1, nch):
                    g.setdefault(off + (cc - 1), []).extend(
                        [lambda c_=cc: P0a(c_), lambda c_=cc: P0b(c_)])
                scheds[f"gen{off}c"] = g
            sched = scheds[KNOB.get("sched", "B")]
            if nch > 2 and KNOB.get("sched", "B") in ("B", "C", "D", "E"):
                sched = scheds["gen1c"]  # legacy scheds only emit chunk 1
            if True:
                for k in range(ntiles + nstage - 1):
                    for fn_ in sched.get(k, ()):
                        fn_()
                    sorder = (range(nstage - 1, -1, -1) if KNOB["order"] == "rev"
                              else range(nstage))
                    for sidx in sorder:
                        t = k - sidx
                        if 0 <= t < ntiles:
                            if sidx == 0 and KNOB.get("prefetch_all", False):
                                continue
                            stages[sidx](t)

            if not DUMP:
                nc.sync.dma_start(
                    out=out, in_=accT.rearrange("p t x -> p (t x)")
                )

    nc.compile()
    return nc


_CACHE = {}


def _get_nc():
    if "nc" not in _CACHE:
        _CACHE["nc"] = build_nc()
    return _CACHE["nc"]


def combine_partials(outs, dumps=None):
    """outs: list of (P, ntiles*NSLOT) arrays -> (cd_sum, l1_sum) float64."""
    tot_min = 0.0
    tot_sqa = 0.0
    tot_sqb = 0.0
    tot_l1 = 0.0
    for o in outs:
        o = o.astype(np.float64).reshape(P, -1, NSLOT)
        tot_min += o[:, :, MINSUM].sum() + o[:, :, T2SUM].sum()
        tot_sqa += o[:, :, SQA].sum()
        tot_sqb += o[:, :, SQB].sum()
        tot_l1 += o[:, :, L1].sum()
    if dumps is not None:
        for d in dumps:
            d = d.astype(np.float64).reshape(P, -1, 19)
            tot_min += d[:, :, 0:16].sum()
            tot_l1 += np.abs(d[:, :, 16:19]).sum()
    cd_sum = tot_min + 2.0 * tot_sqa + tot_sqb
    return cd_sum, tot_l1


def kernel(pred_pts, pred_size, gt_box):
    from concourse.bass_utils import run_bass_kernel_spmd

    pred_pts = np.asarray(pred_pts, dtype=np.float32)
    pred_size = np.asarray(pred_size, dtype=np.float32)
    gt_box = np.asarray(gt_box, dtype=np.float32)

    N = pred_pts.shape[0]
    assert N == N_TOTAL, f"expected {N_TOTAL} rows, got {N}"
    gt12 = gt_box.reshape(N, 12).astype(np.float16)
    gtd_h = np.ascontiguousarray(gt12[:, 6:12])
    gts_h = np.ascontiguousarray(gt12[:, 3:6])
    pred = np.ascontiguousarray(pred_pts.reshape(N, 24).astype(np.float16))
    ps = np.ascontiguousarray(pred_size.astype(np.float16))

    in_maps = [
        {
            "gtd": gtd_h[i * NC_N : (i + 1) * NC_N],
            "gts": gts_h[i * NC_N : (i + 1) * NC_N],
            "pred": pred[i * NC_N : (i + 1) * NC_N],
            "ps": ps[i * NC_N : (i + 1) * NC_N],
        }
        for i in range(N_CORES)
    ]
    res = run_bass_kernel_spmd(_get_nc(), in_maps, core_ids=list(range(N_CORES)))
    dumps = None
    if KNOB.get("dump_last", False):
        dn = KNOB.get("dump_n", 1)
        ntiles = len(TILE_SIZES)
        dumps = []
        for r in res.results:
            for td in range(ntiles - dn, ntiles):
                dumps.append(r[f"outd{td}"])
    cd_sum, l1_sum = combine_partials([r["out"] for r in res.results], dumps)
    cd = cd_sum / (N * 8)
    l1 = l1_sum / (N * 3)
    return np.array([cd, l1], dtype=np.float32)

